# revision 17
# baseline (speedup 1.0000x reference)
"""GateRow kernel for Trainium2 (8 NeuronCores, SPMD gate-parallel).

Problem: out[b, g] = gates[g, 2*x[b, c0[g]] + x[b, c1[g]]]
  x: [16384, 8192] bool, gates: [8192, 4] bool, choices: [8192, 2] int32.

Strategy: bit-pack the batch dimension (8 rows/byte; stored as uint16
words for 2x DVE throughput) so every boolean gate evaluates bitwise.
Every 2-input boolean function is either a single table row (copies,
constants, inverses -- the table holds x, ~x, zeros, ones) or  P op Q
with op in {AND, OR, XOR} and P, Q table rows.  Gates are sharded
across the 8 cores and sorted by op-class into 128-gate blocks so each
core runs one bitwise tensor_tensor per AND/OR/XOR block and nothing at
all for COPY blocks (gathered rows stream straight back out).

Per core: dma_gather ~1.7k rows of 2048B (~3.3 MiB), ~5 DVE bitwise
ops over [128, 1024] uint16 tiles, DMA out 2 MiB of packed results.
Host side: pack bits, build the table, classify/sort gates, unpack +
transpose the packed output.
"""

import sys

for _p in ("/opt/trn_rl_repo", "/opt/pypackages"):
    if _p not in sys.path:
        sys.path.append(_p)

from contextlib import ExitStack

import numpy as np

import concourse.bacc as bacc
import concourse.tile as tile
import concourse.mybir as mybir
from concourse.bass_utils import run_bass_kernel_spmd

B, N, G, NCORES = 16384, 8192, 8192, 8
GPC = G // NCORES          # 1024 gate slots per core
PB = B // 8                # 2048 packed bytes per table row
PW = PB // 2               # 1024 uint16 words per table row
NTAB = 2 * N + 2           # x rows, ~x rows, zeros row, ones row
NBLK = GPC // 128          # 8 blocks of 128 gates per core

# ---------------------------------------------------------------------------
# Gate classification.  Truth table tt (bit i = gates[g, i], i = 2a+b).
# Classes: 0:AND  1:OR  2:XOR (two rows)   3:COPY (single row).
# Sections: 0:x[c0] 1:~x[c0] 2:x[c1] 3:~x[c1] 4:zeros 5:ones.
# ---------------------------------------------------------------------------


def _forms():
    forms = [[None] * 4 for _ in range(16)]
    for tt in range(16):
        for cls in range(3):
            for ps in range(6):
                for qs in range(6):
                    ok = True
                    for a in (0, 1):
                        for b in (0, 1):
                            va = (a, 1 - a, b, 1 - b, 0, 1)[ps]
                            vb = (a, 1 - a, b, 1 - b, 0, 1)[qs]
                            f = (va & vb, va | vb, va ^ vb)[cls]
                            if f != ((tt >> (2 * a + b)) & 1):
                                ok = False
                    if ok and forms[tt][cls] is None:
                        forms[tt][cls] = (ps, qs)
        for ps in range(6):
            ok = all(
                (a, 1 - a, b, 1 - b, 0, 1)[ps] == ((tt >> (2 * a + b)) & 1)
                for a in (0, 1)
                for b in (0, 1)
            )
            if ok and forms[tt][3] is None:
                forms[tt][3] = (ps, ps)
    return forms


_FORMS = _forms()


def _sec_rows(sec, c0, c1):
    return np.select(
        [sec == 0, sec == 1, sec == 2, sec == 3, sec == 4, sec == 5],
        [c0, N + c0, c1, N + c1,
         np.full(sec.shape, 2 * N), np.full(sec.shape, 2 * N + 1)],
    )


# ---------------------------------------------------------------------------
# Device program.  Uniform across cores: blocks [0, na) AND, [na, na+no)
# OR, [na+no, naox) XOR, [naox, 8) COPY.  Gathers run per half (blocks
# 0-3 then 4-7): a p-call for all 4 blocks, a q-call for the AOX blocks.
# ---------------------------------------------------------------------------


def build_nc(na, no, nx):
    naox = na + no + nx
    assert naox <= NBLK
    nc = bacc.Bacc(
        "TRN2", target_bir_lowering=False, debug=False, num_devices=NCORES,
        num_swdge_queues=3,
    )
    ncols = NBLK * 8 + naox * 8  # int16 idx columns: p-stream then q-stream
    tab = nc.dram_tensor("tab", [NTAB, PW], mybir.dt.uint16, kind="ExternalInput")
    idxs = nc.dram_tensor("idxs", [128, ncols], mybir.dt.int16, kind="ExternalInput")
    outd = nc.dram_tensor("out", [GPC, PW], mybir.dt.uint16, kind="ExternalOutput")

    ops = (
        [mybir.AluOpType.bitwise_and] * na
        + [mybir.AluOpType.bitwise_or] * no
        + [mybir.AluOpType.bitwise_xor] * nx
    )
    hb = NBLK // 2  # blocks per half

    with tile.TileContext(nc) as tc, ExitStack() as ctx:
        pconst = ctx.enter_context(tc.tile_pool(name="const", bufs=1))
        pdata = ctx.enter_context(tc.tile_pool(name="data", bufs=1))

        idx_t = pconst.tile([128, ncols], mybir.dt.int16)
        nc.sync.dma_start(idx_t[:], idxs[:])

        comb = pdata.tile([128, NBLK + naox, PW], mybir.dt.uint16)
        lut = pdata.tile([128, max(naox, 1), PW], mybir.dt.uint16)

        # comb slice layout: [q0..q{naox-1} | pA0..pA{naox-1} | pC0..pC{ncopy-1}]
        ncopy = NBLK - naox
        nQ = naox * 128
        nC = ncopy * 128
        regQ = nc.gpsimd.to_reg(nQ)
        regC = nc.gpsimd.to_reg(nC) if nC else None

        nc.gpsimd.dma_gather(
            comb[:, :naox, :],
            tab[:],
            idx_t[:, : nQ // 16],
            nQ,
            regQ,
            PW,
            single_packet=True,
        )
        nc.gpsimd.dma_gather(
            comb[:, naox : 2 * naox, :],
            tab[:],
            idx_t[:, nQ // 16 : 2 * nQ // 16],
            nQ,
            regQ,
            PW,
            single_packet=True,
            queue_num=1,
        )
        for j in range(naox):
            nc.vector.tensor_tensor(
                lut[:, j, :], comb[:, naox + j, :], comb[:, j, :], ops[j]
            )
            nc.sync.dma_start(outd[j * 128 : (j + 1) * 128, :], lut[:, j, :])
        if nC:
            nc.gpsimd.dma_gather(
                comb[:, 2 * naox :, :],
                tab[:],
                idx_t[:, 2 * nQ // 16 : (2 * nQ + nC) // 16],
                nC,
                regC,
                PW,
                single_packet=True,
                queue_num=2,
            )
            for jj in range(ncopy):
                j = naox + jj
                nc.sync.dma_start(
                    outd[j * 128 : (j + 1) * 128, :], comb[:, 2 * naox + jj, :]
                )
    nc.compile()
    return nc


_NC_CACHE = {}


def _get_nc(key):
    if key not in _NC_CACHE:
        _NC_CACHE[key] = build_nc(*key)
    return _NC_CACHE[key]


# ---------------------------------------------------------------------------
# Host-side planning.
# ---------------------------------------------------------------------------


def _plan(gates, choices):
    gates8 = np.asarray(gates, dtype=np.uint8)
    ch = np.asarray(choices, dtype=np.int64)
    tt = (gates8 << np.arange(4, dtype=np.uint8)).sum(axis=1).astype(np.int64)

    copyable = np.array([_FORMS[t][3] is not None for t in range(16)])[tt]
    cls_strict = np.array(
        [next(c for c in range(3) if _FORMS[t][c] is not None) for t in range(16)]
    )[tt]
    strict = [np.where(~copyable & (cls_strict == c))[0] for c in range(3)]
    copies = np.where(copyable)[0]

    # deal strict gates round-robin
    assign = [[[] for _ in range(4)] for _ in range(NCORES)]
    for c in range(3):
        for i, g in enumerate(strict[c]):
            assign[i % NCORES][c].append(g)

    maxc = [max(len(assign[k][c]) for k in range(NCORES)) for c in range(3)]
    na, no, nx = (int(np.ceil(m / 128)) for m in maxc)
    naox = na + no + nx
    assert naox <= NBLK, (na, no, nx)
    caps = [na * 128, no * 128, nx * 128]

    # copy-capable gates: pad AOX segments to caps, rest go to COPY blocks
    ci = 0
    copies = list(copies)
    for k in range(NCORES):
        for c in range(3):
            while len(assign[k][c]) < caps[c]:
                assign[k][c].append(copies[ci])
                ci += 1
        need = GPC - naox * 128
        assign[k][3] = copies[ci : ci + need]
        ci += need
    assert ci == len(copies)

    psec_tab = np.full((16, 4), -1, dtype=np.int64)
    qsec_tab = np.full((16, 4), -1, dtype=np.int64)
    for t in range(16):
        for c in range(4):
            if _FORMS[t][c] is not None:
                psec_tab[t, c], qsec_tab[t, c] = _FORMS[t][c]

    g_of_slot = np.empty((NCORES, GPC), dtype=np.int64)
    idx_maps = []
    hb = NBLK // 2
    ncopy = NBLK - naox
    for k in range(NCORES):
        segs, segcls = [], []
        for c in range(4):
            gk = np.asarray(assign[k][c], dtype=np.int64)
            if not len(gk):
                continue
            # sort by p-row for DRAM locality
            pr = _sec_rows(psec_tab[tt[gk], c], ch[gk, 0], ch[gk, 1])
            o = np.argsort(pr, kind="stable")
            segs.append(gk[o])
            segcls.append(np.full(len(gk), c))
        gk = np.concatenate(segs)
        cls = np.concatenate(segcls)
        assert gk.shape == (GPC,)
        g_of_slot[k] = gk
        p_rows = _sec_rows(psec_tab[tt[gk], cls], ch[gk, 0], ch[gk, 1])
        q_rows = _sec_rows(qsec_tab[tt[gk], cls], ch[gk, 0], ch[gk, 1])

        cols = []
        for arr in (q_rows[: naox * 128], p_rows[: naox * 128],
                    p_rows[naox * 128 :]):
            flat = arr.astype(np.int16)
            cols.append(np.tile(flat.reshape(-1, 16).T, (8, 1)))
        idx_maps.append(np.ascontiguousarray(np.concatenate(cols, axis=1)))

    return (na, no, nx), g_of_slot, idx_maps


def _build_tab(x):
    x8 = np.asarray(x, dtype=np.uint8)
    xp = np.packbits(x8, axis=0)              # [PB, N]
    tab = np.empty((NTAB, PB), dtype=np.uint8)
    tab[:N] = xp.T
    tab[N : 2 * N] = 255 - tab[:N]
    tab[2 * N] = 0
    tab[2 * N + 1] = 255
    return tab.view(np.uint16)


# ---------------------------------------------------------------------------
# Entry point
# ---------------------------------------------------------------------------

_PLAN_CACHE = {}


def _get_plan(gates, choices):
    h = hash((gates.tobytes(), choices.tobytes()))
    if h not in _PLAN_CACHE:
        _PLAN_CACHE[h] = _plan(gates, choices)
    return _PLAN_CACHE[h]


def kernel(x, gates, choices):
    aox, g_of_slot, idx_maps = _get_plan(np.asarray(gates), np.asarray(choices))
    tab = _build_tab(x)
    nc = _get_nc(aox)
    in_maps = [{"tab": tab, "idxs": idx_maps[k]} for k in range(NCORES)]
    res = run_bass_kernel_spmd(nc, in_maps, list(range(NCORES)))

    packed = np.empty((G, PB), dtype=np.uint8)
    for k in range(NCORES):
        packed[g_of_slot[k]] = res.results[k]["out"].view(np.uint8)
    out = np.unpackbits(np.ascontiguousarray(packed.T), axis=0)
    return out.view(np.bool_)


# revision 20
# speedup vs baseline: 1.0293x; 1.0293x over previous
"""GateRow kernel for Trainium2 (8 NeuronCores, SPMD gate-parallel).

Problem: out[b, g] = gates[g, 2*x[b, c0[g]] + x[b, c1[g]]]
  x: [16384, 8192] bool, gates: [8192, 4] bool, choices: [8192, 2] int32.

Strategy: bit-pack the batch dimension (8 rows/byte; stored as uint16
words for 2x DVE throughput) so every boolean gate evaluates bitwise.
Every 2-input boolean function is either a single table row (copies,
constants, inverses -- the table holds x, ~x, zeros, ones) or  P op Q
with op in {AND, OR, XOR} and P, Q table rows.  Gates are sharded
across the 8 cores and sorted by op-class into 128-gate blocks so each
core runs one bitwise tensor_tensor per AND/OR/XOR block and nothing at
all for COPY blocks (gathered rows stream straight back out).

Per core: dma_gather ~1.7k rows of 2048B (~3.3 MiB), ~5 DVE bitwise
ops over [128, 1024] uint16 tiles, DMA out 2 MiB of packed results.
Host side: pack bits, build the table, classify/sort gates, unpack +
transpose the packed output.
"""

import sys

for _p in ("/opt/trn_rl_repo", "/opt/pypackages"):
    if _p not in sys.path:
        sys.path.append(_p)

from contextlib import ExitStack

import numpy as np

import concourse.bacc as bacc
import concourse.tile as tile
import concourse.mybir as mybir
from concourse.bass_utils import run_bass_kernel_spmd

B, N, G, NCORES = 16384, 8192, 8192, 8
GPC = G // NCORES          # 1024 gate slots per core
PB = B // 8                # 2048 packed bytes per table row
PW = PB // 2               # 1024 uint16 words per table row
NTAB = 2 * N + 2           # x rows, ~x rows, zeros row, ones row
NBLK = GPC // 128          # 8 blocks of 128 gates per core

# ---------------------------------------------------------------------------
# Gate classification.  Truth table tt (bit i = gates[g, i], i = 2a+b).
# Classes: 0:AND  1:OR  2:XOR (two rows)   3:COPY (single row).
# Sections: 0:x[c0] 1:~x[c0] 2:x[c1] 3:~x[c1] 4:zeros 5:ones.
# ---------------------------------------------------------------------------


def _forms():
    forms = [[None] * 4 for _ in range(16)]
    for tt in range(16):
        for cls in range(3):
            for ps in range(6):
                for qs in range(6):
                    ok = True
                    for a in (0, 1):
                        for b in (0, 1):
                            va = (a, 1 - a, b, 1 - b, 0, 1)[ps]
                            vb = (a, 1 - a, b, 1 - b, 0, 1)[qs]
                            f = (va & vb, va | vb, va ^ vb)[cls]
                            if f != ((tt >> (2 * a + b)) & 1):
                                ok = False
                    if ok and forms[tt][cls] is None:
                        forms[tt][cls] = (ps, qs)
        for ps in range(6):
            ok = all(
                (a, 1 - a, b, 1 - b, 0, 1)[ps] == ((tt >> (2 * a + b)) & 1)
                for a in (0, 1)
                for b in (0, 1)
            )
            if ok and forms[tt][3] is None:
                forms[tt][3] = (ps, ps)
    return forms


_FORMS = _forms()


def _sec_rows(sec, c0, c1):
    return np.select(
        [sec == 0, sec == 1, sec == 2, sec == 3, sec == 4, sec == 5],
        [c0, N + c0, c1, N + c1,
         np.full(sec.shape, 2 * N), np.full(sec.shape, 2 * N + 1)],
    )


# ---------------------------------------------------------------------------
# Device program.  Uniform across cores: blocks [0, na) AND, [na, na+no)
# OR, [na+no, naox) XOR, [naox, 8) COPY.  Gathers run per half (blocks
# 0-3 then 4-7): a p-call for all 4 blocks, a q-call for the AOX blocks.
# ---------------------------------------------------------------------------


def build_nc(na, no, nx):
    naox = na + no + nx
    assert naox <= NBLK
    nc = bacc.Bacc(
        "TRN2", target_bir_lowering=False, debug=False, num_devices=NCORES,
        num_swdge_queues=2,
    )
    ncols = NBLK * 8 + naox * 8  # int16 idx columns: p-stream then q-stream
    tab = nc.dram_tensor("tab", [NTAB, PW], mybir.dt.uint16, kind="ExternalInput")
    idxs = nc.dram_tensor("idxs", [128, ncols], mybir.dt.int16, kind="ExternalInput")
    outd = nc.dram_tensor("out", [GPC, PW], mybir.dt.uint16, kind="ExternalOutput")

    ops = (
        [mybir.AluOpType.bitwise_and] * na
        + [mybir.AluOpType.bitwise_or] * no
        + [mybir.AluOpType.bitwise_xor] * nx
    )
    hb = NBLK // 2  # blocks per half

    with tile.TileContext(nc) as tc, ExitStack() as ctx:
        pconst = ctx.enter_context(tc.tile_pool(name="const", bufs=1))
        pdata = ctx.enter_context(tc.tile_pool(name="data", bufs=1))

        # dummy gather: zeroed indices fetch row 0; exists only to trigger
        # the gpsimd gather-library load before the idx DMA needs the rings
        idx0 = pconst.tile([128, 8], mybir.dt.int16)
        dum = pconst.tile([128, 1, PW], mybir.dt.uint16)
        nc.gpsimd.memset(idx0[:], 0)
        nc.gpsimd.dma_gather(
            dum[:], tab[:], idx0[:], 128, 128, PW, single_packet=True
        )
        idx_t = pconst.tile([128, ncols], mybir.dt.int16)
        nc.sync.dma_start(idx_t[:], idxs[:])

        comb = pdata.tile([128, NBLK + naox, PW], mybir.dt.uint16)
        lut = pdata.tile([128, max(naox, 1), PW], mybir.dt.uint16)

        # comb slice layout: [q0..q{naox-1} | pA0..pA{naox-1} | pC0..pC{ncopy-1}]
        ncopy = NBLK - naox
        nQ = naox * 128
        nC = ncopy * 128
        regQ = nc.gpsimd.to_reg(nQ)
        regC = nc.gpsimd.to_reg(nC) if nC else None

        nc.gpsimd.dma_gather(
            comb[:, :naox, :],
            tab[:],
            idx_t[:, : nQ // 16],
            nQ,
            regQ,
            PW,
            single_packet=True,
        )
        nc.gpsimd.dma_gather(
            comb[:, naox : 2 * naox, :],
            tab[:],
            idx_t[:, nQ // 16 : 2 * nQ // 16],
            nQ,
            regQ,
            PW,
            single_packet=True,
            queue_num=1,
        )
        for j in range(naox):
            nc.vector.tensor_tensor(
                lut[:, j, :], comb[:, naox + j, :], comb[:, j, :], ops[j]
            )
            nc.sync.dma_start(outd[j * 128 : (j + 1) * 128, :], lut[:, j, :])
        if nC:
            nc.gpsimd.dma_gather(
                comb[:, 2 * naox :, :],
                tab[:],
                idx_t[:, 2 * nQ // 16 : (2 * nQ + nC) // 16],
                nC,
                regC,
                PW,
                single_packet=True,
            )
            for jj in range(ncopy):
                j = naox + jj
                nc.sync.dma_start(
                    outd[j * 128 : (j + 1) * 128, :], comb[:, 2 * naox + jj, :]
                )
    nc.compile()
    return nc


_NC_CACHE = {}


def _get_nc(key):
    if key not in _NC_CACHE:
        _NC_CACHE[key] = build_nc(*key)
    return _NC_CACHE[key]


# ---------------------------------------------------------------------------
# Host-side planning.
# ---------------------------------------------------------------------------


def _plan(gates, choices):
    gates8 = np.asarray(gates, dtype=np.uint8)
    ch = np.asarray(choices, dtype=np.int64)
    tt = (gates8 << np.arange(4, dtype=np.uint8)).sum(axis=1).astype(np.int64)

    copyable = np.array([_FORMS[t][3] is not None for t in range(16)])[tt]
    cls_strict = np.array(
        [next(c for c in range(3) if _FORMS[t][c] is not None) for t in range(16)]
    )[tt]
    strict = [np.where(~copyable & (cls_strict == c))[0] for c in range(3)]
    copies = np.where(copyable)[0]

    # deal strict gates round-robin
    assign = [[[] for _ in range(4)] for _ in range(NCORES)]
    for c in range(3):
        for i, g in enumerate(strict[c]):
            assign[i % NCORES][c].append(g)

    maxc = [max(len(assign[k][c]) for k in range(NCORES)) for c in range(3)]
    na, no, nx = (int(np.ceil(m / 128)) for m in maxc)
    naox = na + no + nx
    assert naox <= NBLK, (na, no, nx)
    caps = [na * 128, no * 128, nx * 128]

    # copy-capable gates: pad AOX segments to caps, rest go to COPY blocks
    ci = 0
    copies = list(copies)
    for k in range(NCORES):
        for c in range(3):
            while len(assign[k][c]) < caps[c]:
                assign[k][c].append(copies[ci])
                ci += 1
        need = GPC - naox * 128
        assign[k][3] = copies[ci : ci + need]
        ci += need
    assert ci == len(copies)

    psec_tab = np.full((16, 4), -1, dtype=np.int64)
    qsec_tab = np.full((16, 4), -1, dtype=np.int64)
    for t in range(16):
        for c in range(4):
            if _FORMS[t][c] is not None:
                psec_tab[t, c], qsec_tab[t, c] = _FORMS[t][c]

    g_of_slot = np.empty((NCORES, GPC), dtype=np.int64)
    idx_maps = []
    hb = NBLK // 2
    ncopy = NBLK - naox
    for k in range(NCORES):
        segs, segcls = [], []
        for c in range(4):
            gk = np.asarray(assign[k][c], dtype=np.int64)
            if not len(gk):
                continue
            # sort by p-row for DRAM locality
            pr = _sec_rows(psec_tab[tt[gk], c], ch[gk, 0], ch[gk, 1])
            o = np.argsort(pr, kind="stable")
            segs.append(gk[o])
            segcls.append(np.full(len(gk), c))
        gk = np.concatenate(segs)
        cls = np.concatenate(segcls)
        assert gk.shape == (GPC,)
        g_of_slot[k] = gk
        p_rows = _sec_rows(psec_tab[tt[gk], cls], ch[gk, 0], ch[gk, 1])
        q_rows = _sec_rows(qsec_tab[tt[gk], cls], ch[gk, 0], ch[gk, 1])

        cols = []
        for arr in (q_rows[: naox * 128], p_rows[: naox * 128],
                    p_rows[naox * 128 :]):
            flat = arr.astype(np.int16)
            cols.append(np.tile(flat.reshape(-1, 16).T, (8, 1)))
        idx_maps.append(np.ascontiguousarray(np.concatenate(cols, axis=1)))

    return (na, no, nx), g_of_slot, idx_maps


def _build_tab(x):
    x8 = np.asarray(x, dtype=np.uint8)
    xp = np.packbits(x8, axis=0)              # [PB, N]
    tab = np.empty((NTAB, PB), dtype=np.uint8)
    tab[:N] = xp.T
    tab[N : 2 * N] = 255 - tab[:N]
    tab[2 * N] = 0
    tab[2 * N + 1] = 255
    return tab.view(np.uint16)


# ---------------------------------------------------------------------------
# Entry point
# ---------------------------------------------------------------------------

_PLAN_CACHE = {}


def _get_plan(gates, choices):
    h = hash((gates.tobytes(), choices.tobytes()))
    if h not in _PLAN_CACHE:
        _PLAN_CACHE[h] = _plan(gates, choices)
    return _PLAN_CACHE[h]


def kernel(x, gates, choices):
    aox, g_of_slot, idx_maps = _get_plan(np.asarray(gates), np.asarray(choices))
    tab = _build_tab(x)
    nc = _get_nc(aox)
    in_maps = [{"tab": tab, "idxs": idx_maps[k]} for k in range(NCORES)]
    res = run_bass_kernel_spmd(nc, in_maps, list(range(NCORES)))

    packed = np.empty((G, PB), dtype=np.uint8)
    for k in range(NCORES):
        packed[g_of_slot[k]] = res.results[k]["out"].view(np.uint8)
    out = np.unpackbits(np.ascontiguousarray(packed.T), axis=0)
    return out.view(np.bool_)


# revision 21
# speedup vs baseline: 1.1278x; 1.0957x over previous
"""GateRow kernel for Trainium2 (8 NeuronCores, SPMD gate-parallel).

Problem: out[b, g] = gates[g, 2*x[b, c0[g]] + x[b, c1[g]]]
  x: [16384, 8192] bool, gates: [8192, 4] bool, choices: [8192, 2] int32.

Strategy: bit-pack the batch dimension (8 rows/byte; stored as uint16
words for 2x DVE throughput) so every boolean gate evaluates bitwise.
Every 2-input boolean function is either a single table row (copies,
constants, inverses -- the table holds x, ~x, zeros, ones) or  P op Q
with op in {AND, OR, XOR} and P, Q table rows.  Gates are sharded
across the 8 cores and sorted by op-class into 128-gate blocks so each
core runs one bitwise tensor_tensor per AND/OR/XOR block and nothing at
all for COPY blocks (gathered rows stream straight back out).

Per core: dma_gather ~1.7k rows of 2048B (~3.3 MiB), ~5 DVE bitwise
ops over [128, 1024] uint16 tiles, DMA out 2 MiB of packed results.
Host side: pack bits, build the table, classify/sort gates, unpack +
transpose the packed output.
"""

import sys

for _p in ("/opt/trn_rl_repo", "/opt/pypackages"):
    if _p not in sys.path:
        sys.path.append(_p)

from contextlib import ExitStack

import numpy as np

import concourse.bass as bass
import concourse.bacc as bacc
import concourse.tile as tile
import concourse.mybir as mybir
from concourse.bass_utils import run_bass_kernel_spmd

B, N, G, NCORES = 16384, 8192, 8192, 8
GPC = G // NCORES          # 1024 gate slots per core
PB = B // 8                # 2048 packed bytes per table row
PW = PB // 2               # 1024 uint16 words per table row
NTAB = 2 * N + 2           # x rows, ~x rows, zeros row, ones row
NBLK = GPC // 128          # 8 blocks of 128 gates per core

# ---------------------------------------------------------------------------
# Gate classification.  Truth table tt (bit i = gates[g, i], i = 2a+b).
# Classes: 0:AND  1:OR  2:XOR (two rows)   3:COPY (single row).
# Sections: 0:x[c0] 1:~x[c0] 2:x[c1] 3:~x[c1] 4:zeros 5:ones.
# ---------------------------------------------------------------------------


def _forms():
    forms = [[None] * 4 for _ in range(16)]
    for tt in range(16):
        for cls in range(3):
            for ps in range(6):
                for qs in range(6):
                    ok = True
                    for a in (0, 1):
                        for b in (0, 1):
                            va = (a, 1 - a, b, 1 - b, 0, 1)[ps]
                            vb = (a, 1 - a, b, 1 - b, 0, 1)[qs]
                            f = (va & vb, va | vb, va ^ vb)[cls]
                            if f != ((tt >> (2 * a + b)) & 1):
                                ok = False
                    if ok and forms[tt][cls] is None:
                        forms[tt][cls] = (ps, qs)
        for ps in range(6):
            ok = all(
                (a, 1 - a, b, 1 - b, 0, 1)[ps] == ((tt >> (2 * a + b)) & 1)
                for a in (0, 1)
                for b in (0, 1)
            )
            if ok and forms[tt][3] is None:
                forms[tt][3] = (ps, ps)
    return forms


_FORMS = _forms()


def _sec_rows(sec, c0, c1):
    return np.select(
        [sec == 0, sec == 1, sec == 2, sec == 3, sec == 4, sec == 5],
        [c0, N + c0, c1, N + c1,
         np.full(sec.shape, 2 * N), np.full(sec.shape, 2 * N + 1)],
    )


# ---------------------------------------------------------------------------
# Device program.  Uniform across cores: blocks [0, na) AND, [na, na+no)
# OR, [na+no, naox) XOR, [naox, 8) COPY.  Gathers run per half (blocks
# 0-3 then 4-7): a p-call for all 4 blocks, a q-call for the AOX blocks.
# ---------------------------------------------------------------------------


def build_nc(na, no, nx):
    naox = na + no + nx
    assert naox <= NBLK
    nc = bacc.Bacc(
        "TRN2", target_bir_lowering=False, debug=False, num_devices=NCORES
    )
    nsl = NBLK + naox  # comb slices: [q0..q{naox-1} | pA | pC]
    tab = nc.dram_tensor("tab", [NTAB, PW], mybir.dt.uint16, kind="ExternalInput")
    idxs = nc.dram_tensor("idxs", [128, nsl], mybir.dt.int32, kind="ExternalInput")
    outd = nc.dram_tensor("out", [GPC, PW], mybir.dt.uint16, kind="ExternalOutput")

    ops = (
        [mybir.AluOpType.bitwise_and] * na
        + [mybir.AluOpType.bitwise_or] * no
        + [mybir.AluOpType.bitwise_xor] * nx
    )
    hb = NBLK // 2  # blocks per half

    with tile.TileContext(nc) as tc, ExitStack() as ctx:
        pconst = ctx.enter_context(tc.tile_pool(name="const", bufs=1))
        pdata = ctx.enter_context(tc.tile_pool(name="data", bufs=1))

        idx_t = pconst.tile([128, nsl], mybir.dt.int32)
        nc.sync.dma_start(idx_t[:], idxs[:])

        comb = pdata.tile([128, nsl, PW], mybir.dt.uint16)
        lut = pdata.tile([128, max(naox, 1), PW], mybir.dt.uint16)
        ncopy = NBLK - naox

        def islice(j):
            nc.gpsimd.indirect_dma_start(
                out=comb[:, j, :],
                out_offset=None,
                in_=tab[:],
                in_offset=bass.IndirectOffsetOnAxis(
                    ap=idx_t[:, j : j + 1], axis=0
                ),
            )

        for j in range(2 * naox):
            islice(j)
        for j in range(naox):
            nc.vector.tensor_tensor(
                lut[:, j, :], comb[:, naox + j, :], comb[:, j, :], ops[j]
            )
            nc.sync.dma_start(outd[j * 128 : (j + 1) * 128, :], lut[:, j, :])
        for jj in range(ncopy):
            islice(2 * naox + jj)
            j = naox + jj
            nc.sync.dma_start(
                outd[j * 128 : (j + 1) * 128, :], comb[:, 2 * naox + jj, :]
            )
    nc.compile()
    return nc


_NC_CACHE = {}


def _get_nc(key):
    if key not in _NC_CACHE:
        _NC_CACHE[key] = build_nc(*key)
    return _NC_CACHE[key]


# ---------------------------------------------------------------------------
# Host-side planning.
# ---------------------------------------------------------------------------


def _plan(gates, choices):
    gates8 = np.asarray(gates, dtype=np.uint8)
    ch = np.asarray(choices, dtype=np.int64)
    tt = (gates8 << np.arange(4, dtype=np.uint8)).sum(axis=1).astype(np.int64)

    copyable = np.array([_FORMS[t][3] is not None for t in range(16)])[tt]
    cls_strict = np.array(
        [next(c for c in range(3) if _FORMS[t][c] is not None) for t in range(16)]
    )[tt]
    strict = [np.where(~copyable & (cls_strict == c))[0] for c in range(3)]
    copies = np.where(copyable)[0]

    # deal strict gates round-robin
    assign = [[[] for _ in range(4)] for _ in range(NCORES)]
    for c in range(3):
        for i, g in enumerate(strict[c]):
            assign[i % NCORES][c].append(g)

    maxc = [max(len(assign[k][c]) for k in range(NCORES)) for c in range(3)]
    na, no, nx = (int(np.ceil(m / 128)) for m in maxc)
    naox = na + no + nx
    assert naox <= NBLK, (na, no, nx)
    caps = [na * 128, no * 128, nx * 128]

    # copy-capable gates: pad AOX segments to caps, rest go to COPY blocks
    ci = 0
    copies = list(copies)
    for k in range(NCORES):
        for c in range(3):
            while len(assign[k][c]) < caps[c]:
                assign[k][c].append(copies[ci])
                ci += 1
        need = GPC - naox * 128
        assign[k][3] = copies[ci : ci + need]
        ci += need
    assert ci == len(copies)

    psec_tab = np.full((16, 4), -1, dtype=np.int64)
    qsec_tab = np.full((16, 4), -1, dtype=np.int64)
    for t in range(16):
        for c in range(4):
            if _FORMS[t][c] is not None:
                psec_tab[t, c], qsec_tab[t, c] = _FORMS[t][c]

    g_of_slot = np.empty((NCORES, GPC), dtype=np.int64)
    idx_maps = []
    hb = NBLK // 2
    ncopy = NBLK - naox
    for k in range(NCORES):
        segs, segcls = [], []
        for c in range(4):
            gk = np.asarray(assign[k][c], dtype=np.int64)
            if not len(gk):
                continue
            # sort by p-row for DRAM locality
            pr = _sec_rows(psec_tab[tt[gk], c], ch[gk, 0], ch[gk, 1])
            o = np.argsort(pr, kind="stable")
            segs.append(gk[o])
            segcls.append(np.full(len(gk), c))
        gk = np.concatenate(segs)
        cls = np.concatenate(segcls)
        assert gk.shape == (GPC,)
        g_of_slot[k] = gk
        p_rows = _sec_rows(psec_tab[tt[gk], cls], ch[gk, 0], ch[gk, 1])
        q_rows = _sec_rows(qsec_tab[tt[gk], cls], ch[gk, 0], ch[gk, 1])

        allrows = np.concatenate(
            [q_rows[: naox * 128], p_rows[: naox * 128], p_rows[naox * 128 :]]
        )
        offs = allrows.reshape(-1, 128).T.astype(np.int32)
        idx_maps.append(np.ascontiguousarray(offs))

    return (na, no, nx), g_of_slot, idx_maps


def _build_tab(x):
    x8 = np.asarray(x, dtype=np.uint8)
    xp = np.packbits(x8, axis=0)              # [PB, N]
    tab = np.empty((NTAB, PB), dtype=np.uint8)
    tab[:N] = xp.T
    tab[N : 2 * N] = 255 - tab[:N]
    tab[2 * N] = 0
    tab[2 * N + 1] = 255
    return tab.view(np.uint16)


# ---------------------------------------------------------------------------
# Entry point
# ---------------------------------------------------------------------------

_PLAN_CACHE = {}


def _get_plan(gates, choices):
    h = hash((gates.tobytes(), choices.tobytes()))
    if h not in _PLAN_CACHE:
        _PLAN_CACHE[h] = _plan(gates, choices)
    return _PLAN_CACHE[h]


def kernel(x, gates, choices):
    aox, g_of_slot, idx_maps = _get_plan(np.asarray(gates), np.asarray(choices))
    tab = _build_tab(x)
    nc = _get_nc(aox)
    in_maps = [{"tab": tab, "idxs": idx_maps[k]} for k in range(NCORES)]
    res = run_bass_kernel_spmd(nc, in_maps, list(range(NCORES)))

    packed = np.empty((G, PB), dtype=np.uint8)
    for k in range(NCORES):
        packed[g_of_slot[k]] = res.results[k]["out"].view(np.uint8)
    out = np.unpackbits(np.ascontiguousarray(packed.T), axis=0)
    return out.view(np.bool_)


# revision 22
# speedup vs baseline: 1.1388x; 1.0098x over previous
"""GateRow kernel for Trainium2 (8 NeuronCores, SPMD gate-parallel).

Problem: out[b, g] = gates[g, 2*x[b, c0[g]] + x[b, c1[g]]]
  x: [16384, 8192] bool, gates: [8192, 4] bool, choices: [8192, 2] int32.

Strategy: bit-pack the batch dimension (8 rows/byte; stored as uint16
words for 2x DVE throughput) so every boolean gate evaluates bitwise.
Every 2-input boolean function is either a single table row (copies,
constants, inverses -- the table holds x, ~x, zeros, ones) or  P op Q
with op in {AND, OR, XOR} and P, Q table rows.  Gates are sharded
across the 8 cores and sorted by op-class into 128-gate blocks so each
core runs one bitwise tensor_tensor per AND/OR/XOR block and nothing at
all for COPY blocks (gathered rows stream straight back out).

Per core: dma_gather ~1.7k rows of 2048B (~3.3 MiB), ~5 DVE bitwise
ops over [128, 1024] uint16 tiles, DMA out 2 MiB of packed results.
Host side: pack bits, build the table, classify/sort gates, unpack +
transpose the packed output.
"""

import sys

for _p in ("/opt/trn_rl_repo", "/opt/pypackages"):
    if _p not in sys.path:
        sys.path.append(_p)

from contextlib import ExitStack

import numpy as np

import concourse.bass as bass
import concourse.bacc as bacc
import concourse.tile as tile
import concourse.mybir as mybir
from concourse.bass_utils import run_bass_kernel_spmd

B, N, G, NCORES = 16384, 8192, 8192, 8
GPC = G // NCORES          # 1024 gate slots per core
PB = B // 8                # 2048 packed bytes per table row
PW = PB // 2               # 1024 uint16 words per table row
NTAB = 2 * N + 2           # x rows, ~x rows, zeros row, ones row
NBLK = GPC // 128          # 8 blocks of 128 gates per core

# ---------------------------------------------------------------------------
# Gate classification.  Truth table tt (bit i = gates[g, i], i = 2a+b).
# Classes: 0:AND  1:OR  2:XOR (two rows)   3:COPY (single row).
# Sections: 0:x[c0] 1:~x[c0] 2:x[c1] 3:~x[c1] 4:zeros 5:ones.
# ---------------------------------------------------------------------------


def _forms():
    forms = [[None] * 4 for _ in range(16)]
    for tt in range(16):
        for cls in range(3):
            for ps in range(6):
                for qs in range(6):
                    ok = True
                    for a in (0, 1):
                        for b in (0, 1):
                            va = (a, 1 - a, b, 1 - b, 0, 1)[ps]
                            vb = (a, 1 - a, b, 1 - b, 0, 1)[qs]
                            f = (va & vb, va | vb, va ^ vb)[cls]
                            if f != ((tt >> (2 * a + b)) & 1):
                                ok = False
                    if ok and forms[tt][cls] is None:
                        forms[tt][cls] = (ps, qs)
        for ps in range(6):
            ok = all(
                (a, 1 - a, b, 1 - b, 0, 1)[ps] == ((tt >> (2 * a + b)) & 1)
                for a in (0, 1)
                for b in (0, 1)
            )
            if ok and forms[tt][3] is None:
                forms[tt][3] = (ps, ps)
    return forms


_FORMS = _forms()


def _sec_rows(sec, c0, c1):
    return np.select(
        [sec == 0, sec == 1, sec == 2, sec == 3, sec == 4, sec == 5],
        [c0, N + c0, c1, N + c1,
         np.full(sec.shape, 2 * N), np.full(sec.shape, 2 * N + 1)],
    )


# ---------------------------------------------------------------------------
# Device program.  Uniform across cores: blocks [0, na) AND, [na, na+no)
# OR, [na+no, naox) XOR, [naox, 8) COPY.  Gathers run per half (blocks
# 0-3 then 4-7): a p-call for all 4 blocks, a q-call for the AOX blocks.
# ---------------------------------------------------------------------------


def build_nc(na, no, nx):
    naox = na + no + nx
    assert naox <= NBLK
    nc = bacc.Bacc(
        "TRN2", target_bir_lowering=False, debug=False, num_devices=NCORES
    )
    nsl = NBLK + naox  # comb slices: [q0..q{naox-1} | pA | pC]
    tab = nc.dram_tensor("tab", [NTAB, PW], mybir.dt.uint16, kind="ExternalInput")
    idxs = nc.dram_tensor("idxs", [128, nsl], mybir.dt.int32, kind="ExternalInput")
    outd = nc.dram_tensor("out", [GPC, PW], mybir.dt.uint16, kind="ExternalOutput")

    ops = (
        [mybir.AluOpType.bitwise_and] * na
        + [mybir.AluOpType.bitwise_or] * no
        + [mybir.AluOpType.bitwise_xor] * nx
    )
    hb = NBLK // 2  # blocks per half

    with tile.TileContext(nc) as tc, ExitStack() as ctx:
        pconst = ctx.enter_context(tc.tile_pool(name="const", bufs=1))
        pdata = ctx.enter_context(tc.tile_pool(name="data", bufs=1))

        idx_t = pconst.tile([128, nsl], mybir.dt.int32)
        nc.sync.dma_start(idx_t[:], idxs[:])

        comb = pdata.tile([128, nsl, PW], mybir.dt.uint16)
        lut = pdata.tile([128, max(naox, 1), PW], mybir.dt.uint16)
        ncopy = NBLK - naox

        def islice(j):
            nc.gpsimd.indirect_dma_start(
                out=comb[:, j, :],
                out_offset=None,
                in_=tab[:],
                in_offset=bass.IndirectOffsetOnAxis(
                    ap=idx_t[:, j : j + 1], axis=0
                ),
            )

        for j in range(naox + ncopy):
            islice(j)
        for jj in range(ncopy):
            j = naox + jj
            nc.scalar.dma_start(
                outd[j * 128 : (j + 1) * 128, :], comb[:, naox + jj, :]
            )
        for j in range(naox):
            islice(naox + ncopy + j)
            nc.vector.tensor_tensor(
                lut[:, j, :],
                comb[:, naox + ncopy + j, :],
                comb[:, j, :],
                ops[j],
            )
            nc.sync.dma_start(outd[j * 128 : (j + 1) * 128, :], lut[:, j, :])
    nc.compile()
    return nc


_NC_CACHE = {}


def _get_nc(key):
    if key not in _NC_CACHE:
        _NC_CACHE[key] = build_nc(*key)
    return _NC_CACHE[key]


# ---------------------------------------------------------------------------
# Host-side planning.
# ---------------------------------------------------------------------------


def _plan(gates, choices):
    gates8 = np.asarray(gates, dtype=np.uint8)
    ch = np.asarray(choices, dtype=np.int64)
    tt = (gates8 << np.arange(4, dtype=np.uint8)).sum(axis=1).astype(np.int64)

    copyable = np.array([_FORMS[t][3] is not None for t in range(16)])[tt]
    cls_strict = np.array(
        [next(c for c in range(3) if _FORMS[t][c] is not None) for t in range(16)]
    )[tt]
    strict = [np.where(~copyable & (cls_strict == c))[0] for c in range(3)]
    copies = np.where(copyable)[0]

    # deal strict gates round-robin
    assign = [[[] for _ in range(4)] for _ in range(NCORES)]
    for c in range(3):
        for i, g in enumerate(strict[c]):
            assign[i % NCORES][c].append(g)

    maxc = [max(len(assign[k][c]) for k in range(NCORES)) for c in range(3)]
    na, no, nx = (int(np.ceil(m / 128)) for m in maxc)
    naox = na + no + nx
    assert naox <= NBLK, (na, no, nx)
    caps = [na * 128, no * 128, nx * 128]

    # copy-capable gates: pad AOX segments to caps, rest go to COPY blocks
    ci = 0
    copies = list(copies)
    for k in range(NCORES):
        for c in range(3):
            while len(assign[k][c]) < caps[c]:
                assign[k][c].append(copies[ci])
                ci += 1
        need = GPC - naox * 128
        assign[k][3] = copies[ci : ci + need]
        ci += need
    assert ci == len(copies)

    psec_tab = np.full((16, 4), -1, dtype=np.int64)
    qsec_tab = np.full((16, 4), -1, dtype=np.int64)
    for t in range(16):
        for c in range(4):
            if _FORMS[t][c] is not None:
                psec_tab[t, c], qsec_tab[t, c] = _FORMS[t][c]

    g_of_slot = np.empty((NCORES, GPC), dtype=np.int64)
    idx_maps = []
    hb = NBLK // 2
    ncopy = NBLK - naox
    for k in range(NCORES):
        segs, segcls = [], []
        for c in range(4):
            gk = np.asarray(assign[k][c], dtype=np.int64)
            if not len(gk):
                continue
            # sort by p-row for DRAM locality
            pr = _sec_rows(psec_tab[tt[gk], c], ch[gk, 0], ch[gk, 1])
            o = np.argsort(pr, kind="stable")
            segs.append(gk[o])
            segcls.append(np.full(len(gk), c))
        gk = np.concatenate(segs)
        cls = np.concatenate(segcls)
        assert gk.shape == (GPC,)
        g_of_slot[k] = gk
        p_rows = _sec_rows(psec_tab[tt[gk], cls], ch[gk, 0], ch[gk, 1])
        q_rows = _sec_rows(qsec_tab[tt[gk], cls], ch[gk, 0], ch[gk, 1])

        allrows = np.concatenate(
            [q_rows[: naox * 128], p_rows[naox * 128 :], p_rows[: naox * 128]]
        )
        offs = allrows.reshape(-1, 128).T.astype(np.int32)
        idx_maps.append(np.ascontiguousarray(offs))

    return (na, no, nx), g_of_slot, idx_maps


def _build_tab(x):
    x8 = np.asarray(x, dtype=np.uint8)
    xp = np.packbits(x8, axis=0)              # [PB, N]
    tab = np.empty((NTAB, PB), dtype=np.uint8)
    tab[:N] = xp.T
    tab[N : 2 * N] = 255 - tab[:N]
    tab[2 * N] = 0
    tab[2 * N + 1] = 255
    return tab.view(np.uint16)


# ---------------------------------------------------------------------------
# Entry point
# ---------------------------------------------------------------------------

_PLAN_CACHE = {}


def _get_plan(gates, choices):
    h = hash((gates.tobytes(), choices.tobytes()))
    if h not in _PLAN_CACHE:
        _PLAN_CACHE[h] = _plan(gates, choices)
    return _PLAN_CACHE[h]


def kernel(x, gates, choices):
    aox, g_of_slot, idx_maps = _get_plan(np.asarray(gates), np.asarray(choices))
    tab = _build_tab(x)
    nc = _get_nc(aox)
    in_maps = [{"tab": tab, "idxs": idx_maps[k]} for k in range(NCORES)]
    res = run_bass_kernel_spmd(nc, in_maps, list(range(NCORES)))

    packed = np.empty((G, PB), dtype=np.uint8)
    for k in range(NCORES):
        packed[g_of_slot[k]] = res.results[k]["out"].view(np.uint8)
    out = np.unpackbits(np.ascontiguousarray(packed.T), axis=0)
    return out.view(np.bool_)
